# revision 7
# baseline (speedup 1.0000x reference)
"""Trainium2 Bass kernel for CompactKroneckerFusion.

Math: out = relu(LN((x1@S1 * x2@S2) @ W + b)), where S1/S2 are count-sketch
matrices (exactly one +-1 per row). The product (x1@S1)*(x2@S2) is nonzero
only on sketch buckets hit by BOTH sketches (~117 of 8192 for these shapes),
so the whole computation collapses to small gathers + tiny dense GEMMs:

  J      = {buckets hit by both sketches}            (|J| = nj)
  x1g    = x1 columns that land in J, transposed     [n1, B]
  A1     = (col -> bucket-in-J) +-1 scatter matrix   [n1, nj]
  sk1^T  = A1^T @ x1g                                [nj, B]   (PE matmul)
  ck^T   = sk1^T * sk2^T  (+ ones row for bias)      [nj+1, B] (DVE)
  h      = ck^T^T @ [W[J]; b]                        [B, OUT]  (PE matmul)
  out    = relu((h - mu) * rsqrt(var + eps) * gamma + beta)    (DVE + ACT)

Sharding: data-parallel over batch across 8 cores; A1/A2/W[J] replicated.
Host side only extracts indices / gathers columns (cheap, O(input size));
all FLOPs above run on-device.
"""

import os
import sys
from contextlib import ExitStack

import numpy as np

_REPO = "/opt/trn_rl_repo"
if _REPO not in sys.path:
    sys.path.insert(0, _REPO)

import concourse.bass as bass  # noqa: E402
import concourse.mybir as mybir  # noqa: E402
import concourse.tile as tile  # noqa: E402

N_CORES = 8
PMAX = 128  # partitions / max matmul K and M
NMAX = 512  # max matmul moving free dim (one PSUM bank of f32)
F32 = mybir.dt.float32
LN_EPS = 1e-5

LAST_EXEC_TIME_NS = None
LAST_TRACE_PATH = None


# ---------------------------------------------------------------------------
# Toolchain workaround: this walrus build rejects instructions carrying more
# than one sync wait ("Too many sync wait commands").  After Tile lowering,
# hoist surplus waits onto same-engine NoOps inserted immediately before the
# owning instruction — the engine stalls on the carriers first, so ordering
# semantics are preserved.
# ---------------------------------------------------------------------------
def _split_multi_waits(nc, max_waits=1):
    n_split = 0
    for f in nc.m.functions:
        for blk in f.blocks:
            insts = blk.instructions
            out = []
            for inst in insts:
                si = inst.sync_info
                waits = list(si.on_wait) if si is not None and si.on_wait else []
                if len(waits) > max_waits:
                    extra = waits[: len(waits) - max_waits]
                    si.on_wait[:] = waits[len(waits) - max_waits :]
                    for k, w in enumerate(extra):
                        nop = mybir.InstNoOp(
                            name=f"{inst.name}-wc{k}", ins=[], outs=[]
                        )
                        nop.engine = inst.engine
                        nop.sync_info = mybir.SyncInfo(on_wait=[w], on_update=[])
                        out.append(nop)
                        n_split += 1
                out.append(inst)
            insts[:] = out
    return n_split


# ---------------------------------------------------------------------------
# Host-side restructuring
# ---------------------------------------------------------------------------
def _extract_sketch(S):
    """Count-sketch matrix -> (bucket index, sign) per input dim."""
    S = np.asarray(S, dtype=np.float32)
    idx = np.abs(S).argmax(1).astype(np.int64)
    s = S[np.arange(S.shape[0]), idx]
    return idx, s


def _plan_side(idx, s, pos, jchunks):
    """Group contributing input columns by J-chunk and split into K-subchunks.

    Returns (cols_global_order, per-chunk list of subchunk descriptors).
    Each subchunk: (row_offset_in_xg, A_matrix [len_sub, njc]).
    """
    keep = (s != 0) & (pos[idx] >= 0)
    cols = np.where(keep)[0]
    p = pos[idx[cols]]
    chunk_of = p // PMAX
    order = np.lexsort((cols, chunk_of))
    cols = cols[order]
    p = p[order]
    sg = s[cols]

    per_chunk = []
    row_base = 0
    for ci, (c0, njc) in enumerate(jchunks):
        m = chunk_of[order] == ci
        ccols = np.where(m)[0]  # positions within `cols`
        subs = []
        for s0 in range(0, len(ccols), PMAX):
            sel = ccols[s0 : s0 + PMAX]
            A = np.zeros((len(sel), njc), np.float32)
            A[np.arange(len(sel)), p[sel] - c0] = sg[sel]
            subs.append((row_base + s0, A))
        per_chunk.append(subs)
        row_base += len(ccols)
    return cols, per_chunk


def _prepare(x1, x2, S1, S2, W, b, ln_gamma, ln_beta):
    x1 = np.ascontiguousarray(np.asarray(x1, np.float32))
    x2 = np.ascontiguousarray(np.asarray(x2, np.float32))
    W = np.asarray(W, np.float32)
    b = np.asarray(b, np.float32)
    ln_gamma = np.asarray(ln_gamma, np.float32)
    ln_beta = np.asarray(ln_beta, np.float32)

    B = x1.shape[0]
    OUT = W.shape[1]
    SK = S1.shape[1]
    assert OUT <= NMAX, "OUT dim > 512 not supported by this kernel"
    assert B % (N_CORES * PMAX) == 0

    idx1, s1 = _extract_sketch(S1)
    idx2, s2 = _extract_sketch(S2)
    J = np.intersect1d(idx1[s1 != 0], idx2[s2 != 0])
    nj = len(J)
    pos = np.full(SK, -1, np.int64)
    pos[J] = np.arange(nj)

    jchunks = [(c0, min(PMAX, nj - c0)) for c0 in range(0, nj, PMAX)]

    cols1, sub1 = _plan_side(idx1, s1, pos, jchunks)
    cols2, sub2 = _plan_side(idx2, s2, pos, jchunks)

    x1g = np.ascontiguousarray(x1[:, cols1].T) if len(cols1) else np.zeros(
        (0, B), np.float32
    )
    x2g = np.ascontiguousarray(x2[:, cols2].T) if len(cols2) else np.zeros(
        (0, B), np.float32
    )

    # W rows for each chunk; bias folded in as a ones-row contraction on the
    # last chunk (or as its own chunk when there's no room / no buckets).
    # Compute-engine SBUF writes must start at a 32-aligned partition, so a
    # bias-carrying chunk is padded to 128 rows: ck rows [96:128) are preset
    # to 1.0 (the product overwrites rows up to njc), and Wg rows beyond njc
    # are zero except the bias row at njc — the spurious ones hit zero rows.
    chunks = []
    for ci, (c0, njc) in enumerate(jchunks):
        chunks.append(
            {"njc": njc, "has_bias": False, "nrows": njc,
             "Wg": W[J[c0 : c0 + njc], :], "sub1": sub1[ci], "sub2": sub2[ci]}
        )
    if not chunks or chunks[-1]["njc"] == PMAX:
        chunks.append(
            {"njc": 0, "has_bias": True, "nrows": 0,
             "Wg": np.zeros((0, OUT), np.float32), "sub1": [], "sub2": []}
        )
    ch = chunks[-1]
    ch["has_bias"] = True
    pad = np.zeros((PMAX - ch["njc"], OUT), np.float32)
    pad[0] = b
    ch["Wg"] = np.concatenate([ch["Wg"], pad], 0)
    ch["nrows"] = PMAX
    for ch in chunks:
        ch["Wg"] = np.ascontiguousarray(ch["Wg"], np.float32)

    affine_trivial = bool(np.all(ln_gamma == 1.0) and np.all(ln_beta == 0.0))

    return {
        "B": B,
        "OUT": OUT,
        "B_core": B // N_CORES,
        "n1": x1g.shape[0],
        "n2": x2g.shape[0],
        "x1g": x1g,
        "x2g": x2g,
        "chunks": chunks,
        "affine_trivial": affine_trivial,
        "gvec": np.ascontiguousarray(ln_gamma[None, :]),
        "bvec": np.ascontiguousarray(ln_beta[None, :]),
    }


# ---------------------------------------------------------------------------
# Device program
# ---------------------------------------------------------------------------
def _build_program(plan):
    B_core = plan["B_core"]
    OUT = plan["OUT"]
    chunks = plan["chunks"]
    BT = NMAX if B_core % NMAX == 0 else PMAX
    assert B_core % BT == 0 and BT % PMAX == 0
    n_t = B_core // BT
    n_m = BT // PMAX

    nc = bass.Bass()

    x1g_d = (
        nc.dram_tensor("x1g", [plan["n1"], B_core], F32, kind="ExternalInput")
        if plan["n1"]
        else None
    )
    x2g_d = (
        nc.dram_tensor("x2g", [plan["n2"], B_core], F32, kind="ExternalInput")
        if plan["n2"]
        else None
    )
    a1_d, a2_d, wg_d = [], [], []
    for ci, ch in enumerate(chunks):
        a1_d.append(
            [
                nc.dram_tensor(f"A1_{ci}_{si}", list(A.shape), F32, kind="ExternalInput")
                for si, (_, A) in enumerate(ch["sub1"])
            ]
        )
        a2_d.append(
            [
                nc.dram_tensor(f"A2_{ci}_{si}", list(A.shape), F32, kind="ExternalInput")
                for si, (_, A) in enumerate(ch["sub2"])
            ]
        )
        wg_d.append(
            nc.dram_tensor(f"Wg_{ci}", list(ch["Wg"].shape), F32, kind="ExternalInput")
        )
    if not plan["affine_trivial"]:
        g_d = nc.dram_tensor("gvec", [1, OUT], F32, kind="ExternalInput")
        be_d = nc.dram_tensor("bvec", [1, OUT], F32, kind="ExternalInput")
    y_d = nc.dram_tensor("y", [B_core, OUT], F32, kind="ExternalOutput")

    with tile.TileContext(nc) as tc, ExitStack() as ctx:
        consts = ctx.enter_context(tc.tile_pool(name="consts", bufs=1))
        xin = ctx.enter_context(tc.tile_pool(name="xin", bufs=2))
        ckp = ctx.enter_context(tc.tile_pool(name="ck", bufs=2))
        pss = ctx.enter_context(tc.tile_pool(name="pss", bufs=2, space="PSUM"))
        psh = ctx.enter_context(tc.tile_pool(name="psh", bufs=2, space="PSUM"))
        stat = ctx.enter_context(tc.tile_pool(name="stat", bufs=4))
        outp = ctx.enter_context(tc.tile_pool(name="outp", bufs=3))

        # constants into SBUF once
        a1_sb, a2_sb, wg_sb = [], [], []
        for ci, ch in enumerate(chunks):
            row1 = []
            for si, (_, A) in enumerate(ch["sub1"]):
                t = consts.tile(list(A.shape), F32, tag=f"A1_{ci}_{si}")
                nc.sync.dma_start(out=t[:], in_=a1_d[ci][si][:])
                row1.append(t)
            a1_sb.append(row1)
            row2 = []
            for si, (_, A) in enumerate(ch["sub2"]):
                t = consts.tile(list(A.shape), F32, tag=f"A2_{ci}_{si}")
                nc.sync.dma_start(out=t[:], in_=a2_d[ci][si][:])
                row2.append(t)
            a2_sb.append(row2)
            t = consts.tile(list(ch["Wg"].shape), F32, tag=f"Wg_{ci}")
            nc.sync.dma_start(out=t[:], in_=wg_d[ci][:])
            wg_sb.append(t)
        eps_t = consts.tile([PMAX, 1], F32, tag="eps")
        nc.vector.memset(eps_t[:], LN_EPS)
        if not plan["affine_trivial"]:
            g_sb = consts.tile([PMAX, OUT], F32, tag="gamma")
            nc.sync.dma_start(out=g_sb[:], in_=g_d[:].to_broadcast([PMAX, OUT]))
            be_sb = consts.tile([PMAX, OUT], F32, tag="beta")
            nc.sync.dma_start(out=be_sb[:], in_=be_d[:].to_broadcast([PMAX, OUT]))

        for ti in range(n_t):
            tsl = bass.ts(ti, BT)
            cks = []
            for ci, ch in enumerate(chunks):
                njc = ch["njc"]
                ck = ckp.tile([ch["nrows"], BT], F32, tag=f"ck{ci}")
                if ch["has_bias"]:
                    ones0 = (njc // 32) * 32
                    nc.vector.memset(ck[ones0:PMAX, :], 1.0)
                if njc > 0:
                    ps1 = pss.tile([njc, BT], F32, tag=f"ps1_{ci}")
                    for si, (r0, A) in enumerate(ch["sub1"]):
                        xt = xin.tile([A.shape[0], BT], F32, tag=f"x1_{ci}_{si}")
                        nc.sync.dma_start(
                            out=xt[:], in_=x1g_d[r0 : r0 + A.shape[0], tsl]
                        )
                        nc.tensor.matmul(
                            ps1[:],
                            a1_sb[ci][si][:],
                            xt[:],
                            start=(si == 0),
                            stop=(si == len(ch["sub1"]) - 1),
                        )
                    ps2 = pss.tile([njc, BT], F32, tag=f"ps2_{ci}")
                    for si, (r0, A) in enumerate(ch["sub2"]):
                        xt = xin.tile([A.shape[0], BT], F32, tag=f"x2_{ci}_{si}")
                        nc.sync.dma_start(
                            out=xt[:], in_=x2g_d[r0 : r0 + A.shape[0], tsl]
                        )
                        nc.tensor.matmul(
                            ps2[:],
                            a2_sb[ci][si][:],
                            xt[:],
                            start=(si == 0),
                            stop=(si == len(ch["sub2"]) - 1),
                        )
                    sk1 = ckp.tile([njc, BT], F32, tag=f"sk1_{ci}")
                    nc.scalar.copy(sk1[:], ps1[:])
                    nc.vector.tensor_mul(ck[0:njc, :], sk1[:], ps2[:])
                cks.append(ck)

            for mi in range(n_m):
                msl = bass.ts(mi, PMAX)
                ph = psh.tile([PMAX, OUT], F32, tag="ph")
                for ci, ch in enumerate(chunks):
                    nc.tensor.matmul(
                        ph[:],
                        cks[ci][:, msl],
                        wg_sb[ci][:],
                        start=(ci == 0),
                        stop=(ci == len(chunks) - 1),
                    )
                stats = stat.tile([PMAX, 6], F32, tag="stats")
                nc.vector.bn_stats(stats[:], ph[:])
                mv = stat.tile([PMAX, 2], F32, tag="mv")
                nc.vector.bn_aggr(mv[:], stats[:])
                std = stat.tile([PMAX, 1], F32, tag="std")
                nc.scalar.activation(
                    std[:],
                    mv[:, 1:2],
                    mybir.ActivationFunctionType.Sqrt,
                    bias=eps_t[:],
                )
                rstd = stat.tile([PMAX, 1], F32, tag="rstd")
                nc.vector.reciprocal(rstd[:], std[:])
                nmr = stat.tile([PMAX, 1], F32, tag="nmr")
                nc.vector.tensor_scalar(
                    out=nmr[:],
                    in0=mv[:, 0:1],
                    scalar1=rstd[:],
                    scalar2=-1.0,
                    op0=mybir.AluOpType.mult,
                    op1=mybir.AluOpType.mult,
                )
                out_t = outp.tile([PMAX, OUT], F32, tag="out")
                if plan["affine_trivial"]:
                    nc.scalar.activation(
                        out_t[:],
                        ph[:],
                        mybir.ActivationFunctionType.Relu,
                        bias=nmr[:],
                        scale=rstd[:],
                    )
                else:
                    tmp = outp.tile([PMAX, OUT], F32, tag="tmp")
                    nc.scalar.activation(
                        tmp[:],
                        ph[:],
                        mybir.ActivationFunctionType.Identity,
                        bias=nmr[:],
                        scale=rstd[:],
                    )
                    nc.vector.tensor_mul(tmp[:], tmp[:], g_sb[:])
                    nc.vector.tensor_add(tmp[:], tmp[:], be_sb[:])
                    nc.scalar.activation(
                        out_t[:], tmp[:], mybir.ActivationFunctionType.Relu
                    )
                nc.sync.dma_start(out=y_d[ti * BT + mi * PMAX :][:PMAX, :], in_=out_t[:])

    return nc


# ---------------------------------------------------------------------------
# Entry point
# ---------------------------------------------------------------------------
def kernel(x1, x2, S1, S2, W, b, ln_gamma, ln_beta):
    global LAST_EXEC_TIME_NS, LAST_TRACE_PATH
    plan = _prepare(x1, x2, S1, S2, W, b, ln_gamma, ln_beta)
    nc = _build_program(plan)
    _split_multi_waits(nc)

    common = {}
    for ci, ch in enumerate(plan["chunks"]):
        for si, (_, A) in enumerate(ch["sub1"]):
            common[f"A1_{ci}_{si}"] = A
        for si, (_, A) in enumerate(ch["sub2"]):
            common[f"A2_{ci}_{si}"] = A
        common[f"Wg_{ci}"] = ch["Wg"]
    if not plan["affine_trivial"]:
        common["gvec"] = plan["gvec"]
        common["bvec"] = plan["bvec"]

    B_core = plan["B_core"]
    in_maps = []
    for c in range(N_CORES):
        m = dict(common)
        if plan["n1"]:
            m["x1g"] = np.ascontiguousarray(
                plan["x1g"][:, c * B_core : (c + 1) * B_core]
            )
        if plan["n2"]:
            m["x2g"] = np.ascontiguousarray(
                plan["x2g"][:, c * B_core : (c + 1) * B_core]
            )
        in_maps.append(m)

    trace = os.environ.get("BASS_KERNEL_TRACE", "") == "1"
    kwargs = {}
    if trace:
        from concourse import bass_utils

        bass_utils.upload_artifacts = lambda tmpdir: "local://" + tmpdir
        kwargs["trace"] = True
        if os.environ.get("BASS_KERNEL_TRACE_ALL", "") == "1":
            kwargs["trace_cores"] = list(range(N_CORES))

    from concourse.bass_utils import run_bass_kernel_spmd

    res = run_bass_kernel_spmd(nc, in_maps, list(range(N_CORES)), **kwargs)
    if trace:
        LAST_EXEC_TIME_NS = res.exec_time_ns
        LAST_TRACE_PATH = (
            res.instructions_and_trace[1] if res.instructions_and_trace else None
        )

    return np.concatenate([res.results[c]["y"] for c in range(N_CORES)], 0)


# revision 13
# speedup vs baseline: 1.2851x; 1.2851x over previous
"""Trainium2 Bass kernel for CompactKroneckerFusion.

Math: out = relu(LN((x1@S1 * x2@S2) @ W + b)), where S1/S2 are count-sketch
matrices (exactly one +-1 per row). The product (x1@S1)*(x2@S2) is nonzero
only on sketch buckets hit by BOTH sketches (~117 of 8192 for these shapes),
so the whole computation collapses to small gathers + tiny dense GEMMs:

  J      = {buckets hit by both sketches}            (|J| = nj)
  x1g    = x1 columns that land in J, transposed     [n1, B]
  A1     = (col -> bucket-in-J) +-1 scatter matrix   [n1, nj]
  sk1^T  = A1^T @ x1g                                [nj, B]   (PE matmul)
  ck^T   = sk1^T * sk2^T  (+ ones row for bias)      [nj+1, B] (DVE)
  h      = ck^T^T @ [W[J]; b]                        [B, OUT]  (PE matmul)
  out    = relu((h - mu) * rsqrt(var + eps) * gamma + beta)    (DVE + ACT)

Sharding: data-parallel over batch across 8 cores; A1/A2/W[J] replicated.
Host side only extracts indices / gathers columns (cheap, O(input size));
all FLOPs above run on-device.
"""

import os
import sys
from contextlib import ExitStack

import numpy as np

_REPO = "/opt/trn_rl_repo"
if _REPO not in sys.path:
    sys.path.insert(0, _REPO)

import concourse.bass as bass  # noqa: E402
import concourse.mybir as mybir  # noqa: E402
import concourse.tile as tile  # noqa: E402

N_CORES = 8
PMAX = 128  # partitions / max matmul K and M
NMAX = 512  # max matmul moving free dim (one PSUM bank of f32)
F32 = mybir.dt.float32
LN_EPS = 1e-5

LAST_EXEC_TIME_NS = None
LAST_TRACE_PATH = None
LAST_RESULTS = None


# ---------------------------------------------------------------------------
# Toolchain workaround: this walrus build rejects instructions carrying more
# than one sync wait ("Too many sync wait commands").  After Tile lowering,
# hoist surplus waits onto same-engine NoOps inserted immediately before the
# owning instruction — the engine stalls on the carriers first, so ordering
# semantics are preserved.
# ---------------------------------------------------------------------------
def _split_multi_waits(nc, max_waits=1):
    n_split = 0
    for f in nc.m.functions:
        for blk in f.blocks:
            insts = blk.instructions
            out = []
            for inst in insts:
                si = inst.sync_info
                waits = list(si.on_wait) if si is not None and si.on_wait else []
                if len(waits) > max_waits:
                    extra = waits[: len(waits) - max_waits]
                    si.on_wait[:] = waits[len(waits) - max_waits :]
                    for k, w in enumerate(extra):
                        nop = mybir.InstNoOp(
                            name=f"{inst.name}-wc{k}", ins=[], outs=[]
                        )
                        nop.engine = inst.engine
                        nop.sync_info = mybir.SyncInfo(on_wait=[w], on_update=[])
                        out.append(nop)
                        n_split += 1
                out.append(inst)
            insts[:] = out
    return n_split


# ---------------------------------------------------------------------------
# Host-side restructuring
# ---------------------------------------------------------------------------
def _extract_sketch(S):
    """Count-sketch matrix -> (bucket index, sign) per input dim."""
    S = np.asarray(S, dtype=np.float32)
    idx = np.abs(S).argmax(1).astype(np.int64)
    s = S[np.arange(S.shape[0]), idx]
    return idx, s


def _plan_side(idx, s, pos, jchunks):
    """Group contributing input columns by J-chunk and split into K-subchunks.

    Returns (src_cols, dest_rows, per-chunk subchunk descriptors, n_rows).
    Each subchunk: (row_offset_in_xg, A_matrix [len_sub, njc]).  Chunk row
    bases are 32-aligned so matmul partition reads stay legal.
    """
    keep = (s != 0) & (pos[idx] >= 0)
    cols = np.where(keep)[0]
    p = pos[idx[cols]]
    chunk_of = p // PMAX
    order = np.lexsort((cols, chunk_of))
    cols = cols[order]
    p = p[order]
    sg = s[cols]

    per_chunk = []
    dest_rows = np.empty(len(cols), np.int64)
    row_base = 0
    for ci, (c0, njc) in enumerate(jchunks):
        ccols = np.where(chunk_of[order] == ci)[0]  # positions within `cols`
        dest_rows[ccols] = row_base + np.arange(len(ccols))
        subs = []
        for s0 in range(0, len(ccols), PMAX):
            sel = ccols[s0 : s0 + PMAX]
            A = np.zeros((len(sel), njc), np.float32)
            A[np.arange(len(sel)), p[sel] - c0] = sg[sel]
            subs.append((row_base + s0, A))
        per_chunk.append(subs)
        row_base += len(ccols)
        row_base = (row_base + 31) // 32 * 32
    return cols, dest_rows, per_chunk, max(row_base, 1)


def _prepare(x1, x2, S1, S2, W, b, ln_gamma, ln_beta):
    x1 = np.ascontiguousarray(np.asarray(x1, np.float32))
    x2 = np.ascontiguousarray(np.asarray(x2, np.float32))
    W = np.asarray(W, np.float32)
    b = np.asarray(b, np.float32)
    ln_gamma = np.asarray(ln_gamma, np.float32)
    ln_beta = np.asarray(ln_beta, np.float32)

    B = x1.shape[0]
    OUT = W.shape[1]
    SK = S1.shape[1]
    assert OUT <= NMAX, "OUT dim > 512 not supported by this kernel"
    assert B % (N_CORES * PMAX) == 0

    idx1, s1 = _extract_sketch(S1)
    idx2, s2 = _extract_sketch(S2)
    J = np.intersect1d(idx1[s1 != 0], idx2[s2 != 0])
    nj = len(J)
    pos = np.full(SK, -1, np.int64)
    pos[J] = np.arange(nj)

    jchunks = [(c0, min(PMAX, nj - c0)) for c0 in range(0, nj, PMAX)]

    cols1, dest1, sub1, nr1 = _plan_side(idx1, s1, pos, jchunks)
    cols2, dest2, sub2, nr2 = _plan_side(idx2, s2, pos, jchunks)

    x1g = np.zeros((nr1, B), np.float32)
    if len(cols1):
        x1g[dest1] = x1[:, cols1].T
    x2g = np.zeros((nr2, B), np.float32)
    if len(cols2):
        x2g[dest2] = x2[:, cols2].T

    # W rows for each chunk; bias folded in as a ones-row contraction on the
    # last chunk (or as its own chunk when there's no room / no buckets).
    # Compute-engine SBUF writes must start at a 32-aligned partition, so a
    # bias-carrying chunk is padded to 128 rows: ck rows [96:128) are preset
    # to 1.0 (the product overwrites rows up to njc), and Wg rows beyond njc
    # are zero except the bias row at njc — the spurious ones hit zero rows.
    chunks = []
    for ci, (c0, njc) in enumerate(jchunks):
        chunks.append(
            {"njc": njc, "has_bias": False, "nrows": njc,
             "Wg": W[J[c0 : c0 + njc], :], "sub1": sub1[ci], "sub2": sub2[ci]}
        )
    if not chunks or chunks[-1]["njc"] == PMAX:
        chunks.append(
            {"njc": 0, "has_bias": True, "nrows": 0,
             "Wg": np.zeros((0, OUT), np.float32), "sub1": [], "sub2": []}
        )
    ch = chunks[-1]
    ch["has_bias"] = True
    pad = np.zeros((PMAX - ch["njc"], OUT), np.float32)
    pad[0] = b
    ch["Wg"] = np.concatenate([ch["Wg"], pad], 0)
    ch["nrows"] = PMAX
    for ch in chunks:
        ch["Wg"] = np.ascontiguousarray(ch["Wg"], np.float32)

    affine_trivial = bool(np.all(ln_gamma == 1.0) and np.all(ln_beta == 0.0))

    return {
        "B": B,
        "OUT": OUT,
        "B_core": B // N_CORES,
        "n1": x1g.shape[0],
        "n2": x2g.shape[0],
        "x1g": x1g,
        "x2g": x2g,
        "chunks": chunks,
        "affine_trivial": affine_trivial,
        "gvec": np.ascontiguousarray(ln_gamma[None, :]),
        "bvec": np.ascontiguousarray(ln_beta[None, :]),
    }


# ---------------------------------------------------------------------------
# Device program
# ---------------------------------------------------------------------------
def _build_program(plan):
    B_core = plan["B_core"]
    OUT = plan["OUT"]
    chunks = plan["chunks"]
    BT = NMAX if B_core % NMAX == 0 else PMAX
    assert B_core % BT == 0 and BT % PMAX == 0
    n_t = B_core // BT
    n_m = BT // PMAX

    nc = bass.Bass()

    x1g_d = (
        nc.dram_tensor("x1g", [plan["n1"], B_core], F32, kind="ExternalInput")
        if plan["n1"]
        else None
    )
    x2g_d = (
        nc.dram_tensor("x2g", [plan["n2"], B_core], F32, kind="ExternalInput")
        if plan["n2"]
        else None
    )
    a1_d, a2_d, wg_d = [], [], []
    for ci, ch in enumerate(chunks):
        a1_d.append(
            [
                nc.dram_tensor(f"A1_{ci}_{si}", list(A.shape), F32, kind="ExternalInput")
                for si, (_, A) in enumerate(ch["sub1"])
            ]
        )
        a2_d.append(
            [
                nc.dram_tensor(f"A2_{ci}_{si}", list(A.shape), F32, kind="ExternalInput")
                for si, (_, A) in enumerate(ch["sub2"])
            ]
        )
        wg_d.append(
            nc.dram_tensor(f"Wg_{ci}", list(ch["Wg"].shape), F32, kind="ExternalInput")
        )
    if not plan["affine_trivial"]:
        g_d = nc.dram_tensor("gvec", [1, OUT], F32, kind="ExternalInput")
        be_d = nc.dram_tensor("bvec", [1, OUT], F32, kind="ExternalInput")
    y_d = nc.dram_tensor("y", [B_core, OUT], F32, kind="ExternalOutput")

    with tile.TileContext(nc) as tc, ExitStack() as ctx:
        consts = ctx.enter_context(tc.tile_pool(name="consts", bufs=1))
        xin = ctx.enter_context(tc.tile_pool(name="xin", bufs=1))
        ckp = ctx.enter_context(tc.tile_pool(name="ck", bufs=2))
        pss = ctx.enter_context(tc.tile_pool(name="pss", bufs=2, space="PSUM"))
        psh = ctx.enter_context(tc.tile_pool(name="psh", bufs=4, space="PSUM"))
        stat = ctx.enter_context(tc.tile_pool(name="stat", bufs=4))
        outp = ctx.enter_context(tc.tile_pool(name="outp", bufs=4))

        # Full x panels, one DMA per side (sync and scalar HWDGE rings run in
        # parallel); constants on the gpsimd SWDGE queue, off the critical path.
        x1_sb = None
        if plan["n1"]:
            x1_sb = xin.tile([plan["n1"], B_core], F32, tag="x1")
            nc.sync.dma_start(out=x1_sb[:], in_=x1g_d[:])
        x2_sb = None
        if plan["n2"]:
            x2_sb = xin.tile([plan["n2"], B_core], F32, tag="x2")
            nc.scalar.dma_start(out=x2_sb[:], in_=x2g_d[:])

        a1_sb, a2_sb, wg_sb = [], [], []
        for ci, ch in enumerate(chunks):
            row1 = []
            for si, (_, A) in enumerate(ch["sub1"]):
                t = consts.tile(list(A.shape), F32, tag=f"A1_{ci}_{si}")
                nc.sync.dma_start(out=t[:], in_=a1_d[ci][si][:])
                row1.append(t)
            a1_sb.append(row1)
            row2 = []
            for si, (_, A) in enumerate(ch["sub2"]):
                t = consts.tile(list(A.shape), F32, tag=f"A2_{ci}_{si}")
                nc.scalar.dma_start(out=t[:], in_=a2_d[ci][si][:])
                row2.append(t)
            a2_sb.append(row2)
            t = consts.tile(list(ch["Wg"].shape), F32, tag=f"Wg_{ci}")
            nc.gpsimd.dma_start(out=t[:], in_=wg_d[ci][:])
            wg_sb.append(t)
        eps_t = consts.tile([PMAX, 1], F32, tag="eps")
        nc.vector.memset(eps_t[:], LN_EPS)
        if not plan["affine_trivial"]:
            g_sb = consts.tile([PMAX, OUT], F32, tag="gamma")
            nc.gpsimd.dma_start(out=g_sb[:], in_=g_d[:].to_broadcast([PMAX, OUT]))
            be_sb = consts.tile([PMAX, OUT], F32, tag="beta")
            nc.gpsimd.dma_start(out=be_sb[:], in_=be_d[:].to_broadcast([PMAX, OUT]))

        for ti in range(n_t):
            tsl = bass.ts(ti, BT)
            cks = []
            for ci, ch in enumerate(chunks):
                njc = ch["njc"]
                ck = ckp.tile([ch["nrows"], BT], F32, tag=f"ck{ci}")
                if ch["has_bias"]:
                    ones0 = (njc // 32) * 32
                    nc.vector.memset(ck[ones0:PMAX, :], 1.0)
                if njc > 0:
                    ps1 = pss.tile([njc, BT], F32, tag=f"ps1_{ci}")
                    for si, (r0, A) in enumerate(ch["sub1"]):
                        nc.tensor.matmul(
                            ps1[:],
                            a1_sb[ci][si][:],
                            x1_sb[r0 : r0 + A.shape[0], tsl],
                            start=(si == 0),
                            stop=(si == len(ch["sub1"]) - 1),
                        )
                    ps2 = pss.tile([njc, BT], F32, tag=f"ps2_{ci}")
                    for si, (r0, A) in enumerate(ch["sub2"]):
                        nc.tensor.matmul(
                            ps2[:],
                            a2_sb[ci][si][:],
                            x2_sb[r0 : r0 + A.shape[0], tsl],
                            start=(si == 0),
                            stop=(si == len(ch["sub2"]) - 1),
                        )
                    sk1 = ckp.tile([njc, BT], F32, tag=f"sk1_{ci}")
                    nc.scalar.copy(sk1[:], ps1[:])
                    nc.vector.tensor_mul(ck[0:njc, :], sk1[:], ps2[:])
                cks.append(ck)

            for mi in range(n_m):
                msl = bass.ts(mi, PMAX)
                ph = psh.tile([PMAX, OUT], F32, tag="ph")
                for ci, ch in enumerate(chunks):
                    nc.tensor.matmul(
                        ph[:],
                        cks[ci][:, msl],
                        wg_sb[ci][:],
                        start=(ci == 0),
                        stop=(ci == len(chunks) - 1),
                    )
                stats = stat.tile([PMAX, 6], F32, tag="stats")
                nc.vector.bn_stats(stats[:], ph[:])
                mv = stat.tile([PMAX, 2], F32, tag="mv")
                nc.vector.bn_aggr(mv[:], stats[:])
                std = stat.tile([PMAX, 1], F32, tag="std")
                nc.scalar.activation(
                    std[:],
                    mv[:, 1:2],
                    mybir.ActivationFunctionType.Sqrt,
                    bias=eps_t[:],
                )
                rstd = stat.tile([PMAX, 1], F32, tag="rstd")
                nc.vector.reciprocal(rstd[:], std[:])
                nmr = stat.tile([PMAX, 1], F32, tag="nmr")
                nc.vector.tensor_scalar(
                    out=nmr[:],
                    in0=mv[:, 0:1],
                    scalar1=rstd[:],
                    scalar2=-1.0,
                    op0=mybir.AluOpType.mult,
                    op1=mybir.AluOpType.mult,
                )
                out_t = outp.tile([PMAX, OUT], F32, tag="out")
                if plan["affine_trivial"]:
                    nc.scalar.activation(
                        out_t[:],
                        ph[:],
                        mybir.ActivationFunctionType.Relu,
                        bias=nmr[:],
                        scale=rstd[:],
                    )
                else:
                    tmp = outp.tile([PMAX, OUT], F32, tag="tmp")
                    nc.scalar.activation(
                        tmp[:],
                        ph[:],
                        mybir.ActivationFunctionType.Identity,
                        bias=nmr[:],
                        scale=rstd[:],
                    )
                    nc.vector.tensor_mul(tmp[:], tmp[:], g_sb[:])
                    nc.vector.tensor_add(tmp[:], tmp[:], be_sb[:])
                    nc.scalar.activation(
                        out_t[:], tmp[:], mybir.ActivationFunctionType.Relu
                    )
                st_eng = nc.sync if mi % 2 == 0 else nc.scalar
                st_eng.dma_start(
                    out=y_d[ti * BT + mi * PMAX :][:PMAX, :], in_=out_t[:]
                )

    return nc


# ---------------------------------------------------------------------------
# Entry point
# ---------------------------------------------------------------------------
def kernel(x1, x2, S1, S2, W, b, ln_gamma, ln_beta):
    global LAST_EXEC_TIME_NS, LAST_TRACE_PATH
    plan = _prepare(x1, x2, S1, S2, W, b, ln_gamma, ln_beta)
    nc = _build_program(plan)
    _split_multi_waits(nc)

    common = {}
    for ci, ch in enumerate(plan["chunks"]):
        for si, (_, A) in enumerate(ch["sub1"]):
            common[f"A1_{ci}_{si}"] = A
        for si, (_, A) in enumerate(ch["sub2"]):
            common[f"A2_{ci}_{si}"] = A
        common[f"Wg_{ci}"] = ch["Wg"]
    if not plan["affine_trivial"]:
        common["gvec"] = plan["gvec"]
        common["bvec"] = plan["bvec"]

    B_core = plan["B_core"]
    in_maps = []
    for c in range(N_CORES):
        m = dict(common)
        if plan["n1"]:
            m["x1g"] = np.ascontiguousarray(
                plan["x1g"][:, c * B_core : (c + 1) * B_core]
            )
        if plan["n2"]:
            m["x2g"] = np.ascontiguousarray(
                plan["x2g"][:, c * B_core : (c + 1) * B_core]
            )
        in_maps.append(m)

    trace = os.environ.get("BASS_KERNEL_TRACE", "") == "1"
    kwargs = {}
    if trace:
        from concourse import bass_utils

        bass_utils.upload_artifacts = lambda tmpdir: "local://" + tmpdir
        kwargs["trace"] = True
        if os.environ.get("BASS_KERNEL_TRACE_ALL", "") == "1":
            kwargs["trace_cores"] = list(range(N_CORES))

    from concourse.bass_utils import run_bass_kernel_spmd

    res = run_bass_kernel_spmd(nc, in_maps, list(range(N_CORES)), **kwargs)
    if trace:
        global LAST_RESULTS
        LAST_RESULTS = res
        LAST_EXEC_TIME_NS = res.exec_time_ns
        LAST_TRACE_PATH = (
            res.instructions_and_trace[1] if res.instructions_and_trace else None
        )

    return np.concatenate([res.results[c]["y"] for c in range(N_CORES)], 0)


# revision 17
# speedup vs baseline: 1.3378x; 1.0410x over previous
"""Trainium2 Bass kernel for CompactKroneckerFusion.

Math: out = relu(LN((x1@S1 * x2@S2) @ W + b)), where S1/S2 are count-sketch
matrices (exactly one +-1 per row). The product (x1@S1)*(x2@S2) is nonzero
only on sketch buckets hit by BOTH sketches (~117 of 8192 for these shapes),
so the whole computation collapses to small gathers + tiny dense GEMMs:

  J      = {buckets hit by both sketches}            (|J| = nj)
  x1g    = x1 columns that land in J, transposed     [n1, B]
  A1     = (col -> bucket-in-J) +-1 scatter matrix   [n1, nj]
  sk1^T  = A1^T @ x1g                                [nj, B]   (PE matmul)
  ck^T   = sk1^T * sk2^T  (+ ones row for bias)      [nj+1, B] (DVE)
  h      = ck^T^T @ [W[J]; b]                        [B, OUT]  (PE matmul)
  out    = relu((h - mu) * rsqrt(var + eps) * gamma + beta)    (DVE + ACT)

Sharding: data-parallel over batch across 8 cores; A1/A2/W[J] replicated.
Host side only extracts indices / gathers columns (cheap, O(input size));
all FLOPs above run on-device.
"""

import os
import sys
from contextlib import ExitStack

import numpy as np

_REPO = "/opt/trn_rl_repo"
if _REPO not in sys.path:
    sys.path.insert(0, _REPO)

import concourse.bass as bass  # noqa: E402
import concourse.mybir as mybir  # noqa: E402
import concourse.tile as tile  # noqa: E402

N_CORES = 8
PMAX = 128  # partitions / max matmul K and M
NMAX = 512  # max matmul moving free dim (one PSUM bank of f32)
F32 = mybir.dt.float32
LN_EPS = 1e-5

LAST_EXEC_TIME_NS = None
LAST_TRACE_PATH = None
LAST_RESULTS = None


# ---------------------------------------------------------------------------
# Toolchain workaround: this walrus build rejects instructions carrying more
# than one sync wait ("Too many sync wait commands").  After Tile lowering,
# hoist surplus waits onto same-engine NoOps inserted immediately before the
# owning instruction — the engine stalls on the carriers first, so ordering
# semantics are preserved.
# ---------------------------------------------------------------------------
def _split_multi_waits(nc, max_waits=1):
    n_split = 0
    for f in nc.m.functions:
        for blk in f.blocks:
            insts = blk.instructions
            out = []
            for inst in insts:
                si = inst.sync_info
                waits = list(si.on_wait) if si is not None and si.on_wait else []
                if len(waits) > max_waits:
                    extra = waits[: len(waits) - max_waits]
                    si.on_wait[:] = waits[len(waits) - max_waits :]
                    for k, w in enumerate(extra):
                        nop = mybir.InstNoOp(
                            name=f"{inst.name}-wc{k}", ins=[], outs=[]
                        )
                        nop.engine = inst.engine
                        nop.sync_info = mybir.SyncInfo(on_wait=[w], on_update=[])
                        out.append(nop)
                        n_split += 1
                out.append(inst)
            insts[:] = out
    return n_split


# ---------------------------------------------------------------------------
# Host-side restructuring
# ---------------------------------------------------------------------------
def _extract_sketch(S):
    """Count-sketch matrix -> (bucket index, sign) per input dim."""
    S = np.asarray(S, dtype=np.float32)
    idx = np.abs(S).argmax(1).astype(np.int64)
    s = S[np.arange(S.shape[0]), idx]
    return idx, s


def _plan_side(idx, s, pos, jchunks):
    """Group contributing input columns by J-chunk and split into K-subchunks.

    Returns (src_cols, dest_rows, per-chunk subchunk descriptors, n_rows).
    Each subchunk: (row_offset_in_xg, A_matrix [len_sub, njc]).  Chunk row
    bases are 32-aligned so matmul partition reads stay legal.
    """
    keep = (s != 0) & (pos[idx] >= 0)
    cols = np.where(keep)[0]
    p = pos[idx[cols]]
    chunk_of = p // PMAX
    order = np.lexsort((cols, chunk_of))
    cols = cols[order]
    p = p[order]
    sg = s[cols]

    per_chunk = []
    dest_rows = np.empty(len(cols), np.int64)
    row_base = 0
    for ci, (c0, njc) in enumerate(jchunks):
        ccols = np.where(chunk_of[order] == ci)[0]  # positions within `cols`
        dest_rows[ccols] = row_base + np.arange(len(ccols))
        subs = []
        for s0 in range(0, len(ccols), PMAX):
            sel = ccols[s0 : s0 + PMAX]
            A = np.zeros((len(sel), njc), np.float32)
            A[np.arange(len(sel)), p[sel] - c0] = sg[sel]
            subs.append((row_base + s0, A))
        per_chunk.append(subs)
        row_base += len(ccols)
        row_base = (row_base + 31) // 32 * 32
    return cols, dest_rows, per_chunk, max(row_base, 1)


def _prepare(x1, x2, S1, S2, W, b, ln_gamma, ln_beta):
    x1 = np.ascontiguousarray(np.asarray(x1, np.float32))
    x2 = np.ascontiguousarray(np.asarray(x2, np.float32))
    W = np.asarray(W, np.float32)
    b = np.asarray(b, np.float32)
    ln_gamma = np.asarray(ln_gamma, np.float32)
    ln_beta = np.asarray(ln_beta, np.float32)

    B = x1.shape[0]
    OUT = W.shape[1]
    SK = S1.shape[1]
    assert OUT <= NMAX, "OUT dim > 512 not supported by this kernel"
    assert B % (N_CORES * PMAX) == 0

    idx1, s1 = _extract_sketch(S1)
    idx2, s2 = _extract_sketch(S2)
    J = np.intersect1d(idx1[s1 != 0], idx2[s2 != 0])
    nj = len(J)
    pos = np.full(SK, -1, np.int64)
    pos[J] = np.arange(nj)

    jchunks = [(c0, min(PMAX, nj - c0)) for c0 in range(0, nj, PMAX)]

    cols1, dest1, sub1, nr1 = _plan_side(idx1, s1, pos, jchunks)
    cols2, dest2, sub2, nr2 = _plan_side(idx2, s2, pos, jchunks)

    x1g = np.zeros((nr1, B), np.float32)
    if len(cols1):
        x1g[dest1] = x1[:, cols1].T
    x2g = np.zeros((nr2, B), np.float32)
    if len(cols2):
        x2g[dest2] = x2[:, cols2].T

    # W rows for each chunk; bias folded in as a ones-row contraction on the
    # last chunk (or as its own chunk when there's no room / no buckets).
    # Compute-engine SBUF writes must start at a 32-aligned partition, so a
    # bias-carrying chunk is padded to 128 rows: ck rows [96:128) are preset
    # to 1.0 (the product overwrites rows up to njc), and Wg rows beyond njc
    # are zero except the bias row at njc — the spurious ones hit zero rows.
    chunks = []
    for ci, (c0, njc) in enumerate(jchunks):
        chunks.append(
            {"njc": njc, "has_bias": False, "nrows": njc,
             "Wg": W[J[c0 : c0 + njc], :], "sub1": sub1[ci], "sub2": sub2[ci]}
        )
    if not chunks or chunks[-1]["njc"] == PMAX:
        chunks.append(
            {"njc": 0, "has_bias": True, "nrows": 0,
             "Wg": np.zeros((0, OUT), np.float32), "sub1": [], "sub2": []}
        )
    ch = chunks[-1]
    ch["has_bias"] = True
    pad = np.zeros((PMAX - ch["njc"], OUT), np.float32)
    pad[0] = b
    ch["Wg"] = np.concatenate([ch["Wg"], pad], 0)
    ch["nrows"] = PMAX
    for ch in chunks:
        ch["Wg"] = np.ascontiguousarray(ch["Wg"], np.float32)

    affine_trivial = bool(np.all(ln_gamma == 1.0) and np.all(ln_beta == 0.0))

    return {
        "B": B,
        "OUT": OUT,
        "B_core": B // N_CORES,
        "n1": x1g.shape[0],
        "n2": x2g.shape[0],
        "x1g": x1g,
        "x2g": x2g,
        "chunks": chunks,
        "affine_trivial": affine_trivial,
        "gvec": np.ascontiguousarray(ln_gamma[None, :]),
        "bvec": np.ascontiguousarray(ln_beta[None, :]),
    }


# ---------------------------------------------------------------------------
# Device program
# ---------------------------------------------------------------------------
def _build_program(plan):
    B_core = plan["B_core"]
    OUT = plan["OUT"]
    chunks = plan["chunks"]
    BT = NMAX if B_core % NMAX == 0 else PMAX
    assert B_core % BT == 0 and BT % PMAX == 0
    n_t = B_core // BT
    n_m = BT // PMAX

    nc = bass.Bass()

    x1g_d = (
        nc.dram_tensor("x1g", [plan["n1"], B_core], F32, kind="ExternalInput")
        if plan["n1"]
        else None
    )
    x2g_d = (
        nc.dram_tensor("x2g", [plan["n2"], B_core], F32, kind="ExternalInput")
        if plan["n2"]
        else None
    )
    a1_d, a2_d, wg_d = [], [], []
    for ci, ch in enumerate(chunks):
        a1_d.append(
            [
                nc.dram_tensor(f"A1_{ci}_{si}", list(A.shape), F32, kind="ExternalInput")
                for si, (_, A) in enumerate(ch["sub1"])
            ]
        )
        a2_d.append(
            [
                nc.dram_tensor(f"A2_{ci}_{si}", list(A.shape), F32, kind="ExternalInput")
                for si, (_, A) in enumerate(ch["sub2"])
            ]
        )
        wg_d.append(
            nc.dram_tensor(f"Wg_{ci}", list(ch["Wg"].shape), F32, kind="ExternalInput")
        )
    if not plan["affine_trivial"]:
        g_d = nc.dram_tensor("gvec", [1, OUT], F32, kind="ExternalInput")
        be_d = nc.dram_tensor("bvec", [1, OUT], F32, kind="ExternalInput")
    y_d = nc.dram_tensor("y", [B_core, OUT], F32, kind="ExternalOutput")

    with tile.TileContext(nc) as tc, ExitStack() as ctx:
        consts = ctx.enter_context(tc.tile_pool(name="consts", bufs=1))
        xin = ctx.enter_context(tc.tile_pool(name="xin", bufs=3))
        ckp = ctx.enter_context(tc.tile_pool(name="ck", bufs=2))
        pss = ctx.enter_context(tc.tile_pool(name="pss", bufs=2, space="PSUM"))
        psh = ctx.enter_context(tc.tile_pool(name="psh", bufs=4, space="PSUM"))
        stat = ctx.enter_context(tc.tile_pool(name="stat", bufs=4))
        outp = ctx.enter_context(tc.tile_pool(name="outp", bufs=4))

        a1_sb, a2_sb, wg_sb = [], [], []
        for ci, ch in enumerate(chunks):
            row1 = []
            for si, (_, A) in enumerate(ch["sub1"]):
                t = consts.tile(list(A.shape), F32, tag=f"A1_{ci}_{si}")
                nc.sync.dma_start(out=t[:], in_=a1_d[ci][si][:])
                row1.append(t)
            a1_sb.append(row1)
            row2 = []
            for si, (_, A) in enumerate(ch["sub2"]):
                t = consts.tile(list(A.shape), F32, tag=f"A2_{ci}_{si}")
                nc.scalar.dma_start(out=t[:], in_=a2_d[ci][si][:])
                row2.append(t)
            a2_sb.append(row2)
            t = consts.tile(list(ch["Wg"].shape), F32, tag=f"Wg_{ci}")
            nc.gpsimd.dma_start(out=t[:], in_=wg_d[ci][:])
            wg_sb.append(t)
        eps_t = consts.tile([PMAX, 1], F32, tag="eps")
        nc.vector.memset(eps_t[:], LN_EPS)
        if not plan["affine_trivial"]:
            g_sb = consts.tile([PMAX, OUT], F32, tag="gamma")
            nc.gpsimd.dma_start(out=g_sb[:], in_=g_d[:].to_broadcast([PMAX, OUT]))
            be_sb = consts.tile([PMAX, OUT], F32, tag="beta")
            nc.gpsimd.dma_start(out=be_sb[:], in_=be_d[:].to_broadcast([PMAX, OUT]))

        # x panels stream in pieces of PW batch columns: DMA completion
        # latency is ~1.5-3 us, so small pieces let the PE start early and
        # pipeline transfers behind compute.  x1 rides the sync HWDGE ring,
        # x2 the scalar ring.
        PW = 256
        n_p = BT // PW

        any_buckets = any(ch["njc"] > 0 for ch in chunks)
        for ti in range(n_t):
            cks = []
            for ci, ch in enumerate(chunks):
                ck = ckp.tile([ch["nrows"], BT], F32, tag=f"ck{ci}")
                if ch["has_bias"]:
                    ones0 = (ch["njc"] // 32) * 32
                    nc.vector.memset(ck[ones0:PMAX, :], 1.0)
                cks.append(ck)
            for pi in range(n_p):
                if not any_buckets:
                    break
                psl = bass.ds(ti * BT + pi * PW, PW)
                x1t = xin.tile([plan["n1"], PW], F32, tag="x1")
                nc.sync.dma_start(out=x1t[:], in_=x1g_d[:, psl])
                x2t = xin.tile([plan["n2"], PW], F32, tag="x2")
                nc.scalar.dma_start(out=x2t[:], in_=x2g_d[:, psl])
                for ci, ch in enumerate(chunks):
                    njc = ch["njc"]
                    if njc == 0:
                        continue
                    ps1 = pss.tile([njc, PW], F32, tag=f"ps1_{ci}")
                    for si, (r0, A) in enumerate(ch["sub1"]):
                        nc.tensor.matmul(
                            ps1[:],
                            a1_sb[ci][si][:],
                            x1t[r0 : r0 + A.shape[0], :],
                            start=(si == 0),
                            stop=(si == len(ch["sub1"]) - 1),
                        )
                    ps2 = pss.tile([njc, PW], F32, tag=f"ps2_{ci}")
                    for si, (r0, A) in enumerate(ch["sub2"]):
                        nc.tensor.matmul(
                            ps2[:],
                            a2_sb[ci][si][:],
                            x2t[r0 : r0 + A.shape[0], :],
                            start=(si == 0),
                            stop=(si == len(ch["sub2"]) - 1),
                        )
                    sk1 = ckp.tile([njc, PW], F32, tag=f"sk1_{ci}")
                    nc.scalar.copy(sk1[:], ps1[:])
                    nc.vector.tensor_mul(
                        cks[ci][0:njc, bass.ts(pi, PW)], sk1[:], ps2[:]
                    )

            for mi in range(n_m):
                msl = bass.ts(mi, PMAX)
                ph = psh.tile([PMAX, OUT], F32, tag="ph")
                for ci, ch in enumerate(chunks):
                    nc.tensor.matmul(
                        ph[:],
                        cks[ci][:, msl],
                        wg_sb[ci][:],
                        start=(ci == 0),
                        stop=(ci == len(chunks) - 1),
                    )
                stats = stat.tile([PMAX, 6], F32, tag="stats")
                nc.vector.bn_stats(stats[:], ph[:])
                mv = stat.tile([PMAX, 2], F32, tag="mv")
                nc.vector.bn_aggr(mv[:], stats[:])
                std = stat.tile([PMAX, 1], F32, tag="std")
                nc.scalar.activation(
                    std[:],
                    mv[:, 1:2],
                    mybir.ActivationFunctionType.Sqrt,
                    bias=eps_t[:],
                )
                rstd = stat.tile([PMAX, 1], F32, tag="rstd")
                nc.vector.reciprocal(rstd[:], std[:])
                nmr = stat.tile([PMAX, 1], F32, tag="nmr")
                nc.vector.tensor_scalar(
                    out=nmr[:],
                    in0=mv[:, 0:1],
                    scalar1=rstd[:],
                    scalar2=-1.0,
                    op0=mybir.AluOpType.mult,
                    op1=mybir.AluOpType.mult,
                )
                out_t = outp.tile([PMAX, OUT], F32, tag="out")
                if plan["affine_trivial"]:
                    nc.scalar.activation(
                        out_t[:],
                        ph[:],
                        mybir.ActivationFunctionType.Relu,
                        bias=nmr[:],
                        scale=rstd[:],
                    )
                else:
                    tmp = outp.tile([PMAX, OUT], F32, tag="tmp")
                    nc.scalar.activation(
                        tmp[:],
                        ph[:],
                        mybir.ActivationFunctionType.Identity,
                        bias=nmr[:],
                        scale=rstd[:],
                    )
                    nc.vector.tensor_mul(tmp[:], tmp[:], g_sb[:])
                    nc.vector.tensor_add(tmp[:], tmp[:], be_sb[:])
                    nc.scalar.activation(
                        out_t[:], tmp[:], mybir.ActivationFunctionType.Relu
                    )
                st_eng = nc.sync if mi % 2 == 0 else nc.scalar
                st_eng.dma_start(
                    out=y_d[ti * BT + mi * PMAX :][:PMAX, :], in_=out_t[:]
                )

    return nc


# ---------------------------------------------------------------------------
# Entry point
# ---------------------------------------------------------------------------
def kernel(x1, x2, S1, S2, W, b, ln_gamma, ln_beta):
    global LAST_EXEC_TIME_NS, LAST_TRACE_PATH
    plan = _prepare(x1, x2, S1, S2, W, b, ln_gamma, ln_beta)
    nc = _build_program(plan)
    _split_multi_waits(nc)

    common = {}
    for ci, ch in enumerate(plan["chunks"]):
        for si, (_, A) in enumerate(ch["sub1"]):
            common[f"A1_{ci}_{si}"] = A
        for si, (_, A) in enumerate(ch["sub2"]):
            common[f"A2_{ci}_{si}"] = A
        common[f"Wg_{ci}"] = ch["Wg"]
    if not plan["affine_trivial"]:
        common["gvec"] = plan["gvec"]
        common["bvec"] = plan["bvec"]

    B_core = plan["B_core"]
    in_maps = []
    for c in range(N_CORES):
        m = dict(common)
        if plan["n1"]:
            m["x1g"] = np.ascontiguousarray(
                plan["x1g"][:, c * B_core : (c + 1) * B_core]
            )
        if plan["n2"]:
            m["x2g"] = np.ascontiguousarray(
                plan["x2g"][:, c * B_core : (c + 1) * B_core]
            )
        in_maps.append(m)

    trace = os.environ.get("BASS_KERNEL_TRACE", "") == "1"
    kwargs = {}
    if trace:
        from concourse import bass_utils

        bass_utils.upload_artifacts = lambda tmpdir: "local://" + tmpdir
        kwargs["trace"] = True
        if os.environ.get("BASS_KERNEL_TRACE_ALL", "") == "1":
            kwargs["trace_cores"] = list(range(N_CORES))

    from concourse.bass_utils import run_bass_kernel_spmd

    res = run_bass_kernel_spmd(nc, in_maps, list(range(N_CORES)), **kwargs)
    if trace:
        global LAST_RESULTS
        LAST_RESULTS = res
        LAST_EXEC_TIME_NS = res.exec_time_ns
        LAST_TRACE_PATH = (
            res.instructions_and_trace[1] if res.instructions_and_trace else None
        )

    return np.concatenate([res.results[c]["y"] for c in range(N_CORES)], 0)


# revision 21
# speedup vs baseline: 1.3940x; 1.0420x over previous
"""Trainium2 Bass kernel for CompactKroneckerFusion.

Math: out = relu(LN((x1@S1 * x2@S2) @ W + b)), where S1/S2 are count-sketch
matrices (exactly one +-1 per row). The product (x1@S1)*(x2@S2) is nonzero
only on sketch buckets hit by BOTH sketches (~117 of 8192 for these shapes),
so the whole computation collapses to small gathers + tiny dense GEMMs:

  J      = {buckets hit by both sketches}            (|J| = nj)
  x1g    = x1 columns that land in J, transposed     [n1, B]
  A1     = (col -> bucket-in-J) +-1 scatter matrix   [n1, nj]
  sk1^T  = A1^T @ x1g                                [nj, B]   (PE matmul)
  ck^T   = sk1^T * sk2^T  (+ ones row for bias)      [nj+1, B] (DVE)
  h      = ck^T^T @ [W[J]; b]                        [B, OUT]  (PE matmul)
  out    = relu((h - mu) * rsqrt(var + eps) * gamma + beta)    (DVE + ACT)

Sharding: data-parallel over batch across 8 cores; A1/A2/W[J] replicated.
Host side only extracts indices / gathers columns (cheap, O(input size));
all FLOPs above run on-device.
"""

import os
import sys
from contextlib import ExitStack

import numpy as np

_REPO = "/opt/trn_rl_repo"
if _REPO not in sys.path:
    sys.path.insert(0, _REPO)

import concourse.bass as bass  # noqa: E402
import concourse.mybir as mybir  # noqa: E402
import concourse.tile as tile  # noqa: E402

N_CORES = 8
PMAX = 128  # partitions / max matmul K and M
NMAX = 512  # max matmul moving free dim (one PSUM bank of f32)
F32 = mybir.dt.float32
LN_EPS = 1e-5

# PE fp32 matmul runs at 4 cycles/row; float32r (same 32-bit storage,
# reduced-precision PE path) runs at 1 cycle/row for moving dim >= 256.
# All matmul operands (x panels, A scatter matrices, Wg, ck) are declared
# float32r end-to-end; PSUM accumulation stays fp32.
MM_DT = os.environ.get("BASS_KERNEL_MM_DT", "float32r")
XDT = mybir.dt.float32r if MM_DT == "float32r" else mybir.dt.float32

LAST_EXEC_TIME_NS = None
LAST_TRACE_PATH = None
LAST_RESULTS = None


# ---------------------------------------------------------------------------
# Toolchain workaround: this walrus build rejects instructions carrying more
# than one sync wait ("Too many sync wait commands").  After Tile lowering,
# hoist surplus waits onto same-engine NoOps inserted immediately before the
# owning instruction — the engine stalls on the carriers first, so ordering
# semantics are preserved.
# ---------------------------------------------------------------------------
def _split_multi_waits(nc, max_waits=1):
    n_split = 0
    for f in nc.m.functions:
        for blk in f.blocks:
            insts = blk.instructions
            out = []
            for inst in insts:
                si = inst.sync_info
                waits = list(si.on_wait) if si is not None and si.on_wait else []
                if len(waits) > max_waits:
                    extra = waits[: len(waits) - max_waits]
                    si.on_wait[:] = waits[len(waits) - max_waits :]
                    for k, w in enumerate(extra):
                        nop = mybir.InstNoOp(
                            name=f"{inst.name}-wc{k}", ins=[], outs=[]
                        )
                        nop.engine = inst.engine
                        nop.sync_info = mybir.SyncInfo(on_wait=[w], on_update=[])
                        out.append(nop)
                        n_split += 1
                out.append(inst)
            insts[:] = out
    return n_split


# ---------------------------------------------------------------------------
# Host-side restructuring
# ---------------------------------------------------------------------------
def _extract_sketch(S):
    """Count-sketch matrix -> (bucket index, sign) per input dim."""
    S = np.asarray(S, dtype=np.float32)
    idx = np.abs(S).argmax(1).astype(np.int64)
    s = S[np.arange(S.shape[0]), idx]
    return idx, s


def _plan_side(idx, s, pos, jchunks):
    """Group contributing input columns by J-chunk and split into K-subchunks.

    Returns (src_cols, dest_rows, per-chunk subchunk descriptors, n_rows).
    Each subchunk: (row_offset_in_xg, A_matrix [len_sub, njc]).  Chunk row
    bases are 32-aligned so matmul partition reads stay legal.
    """
    keep = (s != 0) & (pos[idx] >= 0)
    cols = np.where(keep)[0]
    p = pos[idx[cols]]
    chunk_of = p // PMAX
    order = np.lexsort((cols, chunk_of))
    cols = cols[order]
    p = p[order]
    sg = s[cols]

    per_chunk = []
    dest_rows = np.empty(len(cols), np.int64)
    row_base = 0
    for ci, (c0, njc) in enumerate(jchunks):
        ccols = np.where(chunk_of[order] == ci)[0]  # positions within `cols`
        dest_rows[ccols] = row_base + np.arange(len(ccols))
        subs = []
        for s0 in range(0, len(ccols), PMAX):
            sel = ccols[s0 : s0 + PMAX]
            A = np.zeros((len(sel), njc), np.float32)
            A[np.arange(len(sel)), p[sel] - c0] = sg[sel]
            subs.append((row_base + s0, A))
        per_chunk.append(subs)
        row_base += len(ccols)
        row_base = (row_base + 31) // 32 * 32
    return cols, dest_rows, per_chunk, max(row_base, 1)


def _prepare(x1, x2, S1, S2, W, b, ln_gamma, ln_beta):
    x1 = np.ascontiguousarray(np.asarray(x1, np.float32))
    x2 = np.ascontiguousarray(np.asarray(x2, np.float32))
    W = np.asarray(W, np.float32)
    b = np.asarray(b, np.float32)
    ln_gamma = np.asarray(ln_gamma, np.float32)
    ln_beta = np.asarray(ln_beta, np.float32)

    B = x1.shape[0]
    OUT = W.shape[1]
    SK = S1.shape[1]
    assert OUT <= NMAX, "OUT dim > 512 not supported by this kernel"
    assert B % (N_CORES * PMAX) == 0

    idx1, s1 = _extract_sketch(S1)
    idx2, s2 = _extract_sketch(S2)
    J = np.intersect1d(idx1[s1 != 0], idx2[s2 != 0])
    nj = len(J)
    pos = np.full(SK, -1, np.int64)
    pos[J] = np.arange(nj)

    jchunks = [(c0, min(PMAX, nj - c0)) for c0 in range(0, nj, PMAX)]

    cols1, dest1, sub1, nr1 = _plan_side(idx1, s1, pos, jchunks)
    cols2, dest2, sub2, nr2 = _plan_side(idx2, s2, pos, jchunks)

    x1g = np.zeros((nr1, B), np.float32)
    if len(cols1):
        x1g[dest1] = x1[:, cols1].T
    x2g = np.zeros((nr2, B), np.float32)
    if len(cols2):
        x2g[dest2] = x2[:, cols2].T

    # W rows for each chunk; bias folded in as a ones-row contraction on the
    # last chunk (or as its own chunk when there's no room / no buckets).
    # Compute-engine SBUF writes must start at a 32-aligned partition, so a
    # bias-carrying chunk is padded to 128 rows: ck rows [96:128) are preset
    # to 1.0 (the product overwrites rows up to njc), and Wg rows beyond njc
    # are zero except the bias row at njc — the spurious ones hit zero rows.
    chunks = []
    for ci, (c0, njc) in enumerate(jchunks):
        chunks.append(
            {"njc": njc, "has_bias": False, "nrows": njc,
             "Wg": W[J[c0 : c0 + njc], :], "sub1": sub1[ci], "sub2": sub2[ci]}
        )
    if not chunks or chunks[-1]["njc"] == PMAX:
        chunks.append(
            {"njc": 0, "has_bias": True, "nrows": 0,
             "Wg": np.zeros((0, OUT), np.float32), "sub1": [], "sub2": []}
        )
    ch = chunks[-1]
    ch["has_bias"] = True
    pad = np.zeros((PMAX - ch["njc"], OUT), np.float32)
    pad[0] = b
    ch["Wg"] = np.concatenate([ch["Wg"], pad], 0)
    ch["nrows"] = PMAX
    for ch in chunks:
        ch["Wg"] = np.ascontiguousarray(ch["Wg"], np.float32)

    affine_trivial = bool(np.all(ln_gamma == 1.0) and np.all(ln_beta == 0.0))

    return {
        "B": B,
        "OUT": OUT,
        "B_core": B // N_CORES,
        "n1": x1g.shape[0],
        "n2": x2g.shape[0],
        "x1g": x1g,
        "x2g": x2g,
        "chunks": chunks,
        "affine_trivial": affine_trivial,
        "gvec": np.ascontiguousarray(ln_gamma[None, :]),
        "bvec": np.ascontiguousarray(ln_beta[None, :]),
    }


# ---------------------------------------------------------------------------
# Device program
# ---------------------------------------------------------------------------
def _build_program(plan):
    B_core = plan["B_core"]
    OUT = plan["OUT"]
    chunks = plan["chunks"]
    BT = NMAX if B_core % NMAX == 0 else PMAX
    assert B_core % BT == 0 and BT % PMAX == 0
    n_t = B_core // BT
    n_m = BT // PMAX

    nc = bass.Bass()

    x1g_d = (
        nc.dram_tensor("x1g", [plan["n1"], B_core], XDT, kind="ExternalInput")
        if plan["n1"]
        else None
    )
    x2g_d = (
        nc.dram_tensor("x2g", [plan["n2"], B_core], XDT, kind="ExternalInput")
        if plan["n2"]
        else None
    )
    a1_d, a2_d, wg_d = [], [], []
    for ci, ch in enumerate(chunks):
        a1_d.append(
            [
                nc.dram_tensor(f"A1_{ci}_{si}", list(A.shape), XDT, kind="ExternalInput")
                for si, (_, A) in enumerate(ch["sub1"])
            ]
        )
        a2_d.append(
            [
                nc.dram_tensor(f"A2_{ci}_{si}", list(A.shape), XDT, kind="ExternalInput")
                for si, (_, A) in enumerate(ch["sub2"])
            ]
        )
        wg_d.append(
            nc.dram_tensor(f"Wg_{ci}", list(ch["Wg"].shape), XDT, kind="ExternalInput")
        )
    if not plan["affine_trivial"]:
        g_d = nc.dram_tensor("gvec", [1, OUT], F32, kind="ExternalInput")
        be_d = nc.dram_tensor("bvec", [1, OUT], F32, kind="ExternalInput")
    y_d = nc.dram_tensor("y", [B_core, OUT], F32, kind="ExternalOutput")

    with tile.TileContext(nc) as tc, ExitStack() as ctx:
        consts = ctx.enter_context(tc.tile_pool(name="consts", bufs=1))
        xin = ctx.enter_context(tc.tile_pool(name="xin", bufs=3))
        ckp = ctx.enter_context(tc.tile_pool(name="ck", bufs=2))
        pss = ctx.enter_context(tc.tile_pool(name="pss", bufs=2, space="PSUM"))
        psh = ctx.enter_context(tc.tile_pool(name="psh", bufs=4, space="PSUM"))
        stat = ctx.enter_context(tc.tile_pool(name="stat", bufs=4))
        outp = ctx.enter_context(tc.tile_pool(name="outp", bufs=4))

        a1_sb, a2_sb, wg_sb = [], [], []
        for ci, ch in enumerate(chunks):
            row1 = []
            for si, (_, A) in enumerate(ch["sub1"]):
                t = consts.tile(list(A.shape), XDT, tag=f"A1_{ci}_{si}")
                nc.sync.dma_start(out=t[:], in_=a1_d[ci][si][:])
                row1.append(t)
            a1_sb.append(row1)
            row2 = []
            for si, (_, A) in enumerate(ch["sub2"]):
                t = consts.tile(list(A.shape), XDT, tag=f"A2_{ci}_{si}")
                nc.scalar.dma_start(out=t[:], in_=a2_d[ci][si][:])
                row2.append(t)
            a2_sb.append(row2)
            t = consts.tile(list(ch["Wg"].shape), XDT, tag=f"Wg_{ci}")
            nc.gpsimd.dma_start(out=t[:], in_=wg_d[ci][:])
            wg_sb.append(t)
        eps_t = consts.tile([PMAX, 1], F32, tag="eps")
        nc.vector.memset(eps_t[:], LN_EPS)
        if not plan["affine_trivial"]:
            g_sb = consts.tile([PMAX, OUT], F32, tag="gamma")
            nc.gpsimd.dma_start(out=g_sb[:], in_=g_d[:].to_broadcast([PMAX, OUT]))
            be_sb = consts.tile([PMAX, OUT], F32, tag="beta")
            nc.gpsimd.dma_start(out=be_sb[:], in_=be_d[:].to_broadcast([PMAX, OUT]))

        # x panels stream in pieces of PW batch columns: DMA completion
        # latency is ~1.5-3 us, so small pieces let the PE start early and
        # pipeline transfers behind compute.  x1 rides the sync HWDGE ring,
        # x2 the scalar ring.
        PW = 256
        n_p = BT // PW

        any_buckets = any(ch["njc"] > 0 for ch in chunks)
        for ti in range(n_t):
            cks = []
            for ci, ch in enumerate(chunks):
                ck = ckp.tile([ch["nrows"], BT], XDT, tag=f"ck{ci}")
                if ch["has_bias"]:
                    ones0 = (ch["njc"] // 32) * 32
                    nc.vector.memset(ck[ones0:PMAX, :].bitcast(F32), 1.0)
                cks.append(ck)
            for pi in range(n_p):
                if not any_buckets:
                    break
                psl = bass.ds(ti * BT + pi * PW, PW)
                x1t = xin.tile([plan["n1"], PW], XDT, tag="x1")
                nc.sync.dma_start(out=x1t[:], in_=x1g_d[:, psl])
                x2t = xin.tile([plan["n2"], PW], XDT, tag="x2")
                nc.scalar.dma_start(out=x2t[:], in_=x2g_d[:, psl])
                for ci, ch in enumerate(chunks):
                    njc = ch["njc"]
                    if njc == 0:
                        continue
                    ps1 = pss.tile([njc, PW], F32, tag=f"ps1_{ci}")
                    for si, (r0, A) in enumerate(ch["sub1"]):
                        nc.tensor.matmul(
                            ps1[:],
                            a1_sb[ci][si][:],
                            x1t[r0 : r0 + A.shape[0], :],
                            start=(si == 0),
                            stop=(si == len(ch["sub1"]) - 1),
                        )
                    ps2 = pss.tile([njc, PW], F32, tag=f"ps2_{ci}")
                    for si, (r0, A) in enumerate(ch["sub2"]):
                        nc.tensor.matmul(
                            ps2[:],
                            a2_sb[ci][si][:],
                            x2t[r0 : r0 + A.shape[0], :],
                            start=(si == 0),
                            stop=(si == len(ch["sub2"]) - 1),
                        )
                    sk1 = ckp.tile([njc, PW], F32, tag=f"sk1_{ci}")
                    nc.scalar.copy(sk1[:], ps1[:])
                    nc.vector.tensor_mul(
                        cks[ci][0:njc, bass.ts(pi, PW)], sk1[:], ps2[:]
                    )

            for mi in range(n_m):
                msl = bass.ts(mi, PMAX)
                ph = psh.tile([PMAX, OUT], F32, tag="ph")
                for ci, ch in enumerate(chunks):
                    nc.tensor.matmul(
                        ph[:],
                        cks[ci][:, msl],
                        wg_sb[ci][:],
                        start=(ci == 0),
                        stop=(ci == len(chunks) - 1),
                    )
                stats = stat.tile([PMAX, 6], F32, tag="stats")
                nc.vector.bn_stats(stats[:], ph[:])
                mv = stat.tile([PMAX, 2], F32, tag="mv")
                nc.vector.bn_aggr(mv[:], stats[:])
                std = stat.tile([PMAX, 1], F32, tag="std")
                nc.scalar.activation(
                    std[:],
                    mv[:, 1:2],
                    mybir.ActivationFunctionType.Sqrt,
                    bias=eps_t[:],
                )
                rstd = stat.tile([PMAX, 1], F32, tag="rstd")
                nc.vector.reciprocal(rstd[:], std[:])
                nmr = stat.tile([PMAX, 1], F32, tag="nmr")
                nc.vector.tensor_scalar(
                    out=nmr[:],
                    in0=mv[:, 0:1],
                    scalar1=rstd[:],
                    scalar2=-1.0,
                    op0=mybir.AluOpType.mult,
                    op1=mybir.AluOpType.mult,
                )
                out_t = outp.tile([PMAX, OUT], F32, tag="out")
                if plan["affine_trivial"]:
                    nc.scalar.activation(
                        out_t[:],
                        ph[:],
                        mybir.ActivationFunctionType.Relu,
                        bias=nmr[:],
                        scale=rstd[:],
                    )
                else:
                    tmp = outp.tile([PMAX, OUT], F32, tag="tmp")
                    nc.scalar.activation(
                        tmp[:],
                        ph[:],
                        mybir.ActivationFunctionType.Identity,
                        bias=nmr[:],
                        scale=rstd[:],
                    )
                    nc.vector.tensor_mul(tmp[:], tmp[:], g_sb[:])
                    nc.vector.tensor_add(tmp[:], tmp[:], be_sb[:])
                    nc.scalar.activation(
                        out_t[:], tmp[:], mybir.ActivationFunctionType.Relu
                    )
                st_eng = nc.sync if mi % 2 == 0 else nc.scalar
                st_eng.dma_start(
                    out=y_d[ti * BT + mi * PMAX :][:PMAX, :], in_=out_t[:]
                )

    return nc


# ---------------------------------------------------------------------------
# Entry point
# ---------------------------------------------------------------------------
def kernel(x1, x2, S1, S2, W, b, ln_gamma, ln_beta):
    global LAST_EXEC_TIME_NS, LAST_TRACE_PATH
    plan = _prepare(x1, x2, S1, S2, W, b, ln_gamma, ln_beta)
    nc = _build_program(plan)
    _split_multi_waits(nc)

    common = {}
    for ci, ch in enumerate(plan["chunks"]):
        for si, (_, A) in enumerate(ch["sub1"]):
            common[f"A1_{ci}_{si}"] = A
        for si, (_, A) in enumerate(ch["sub2"]):
            common[f"A2_{ci}_{si}"] = A
        common[f"Wg_{ci}"] = ch["Wg"]
    if not plan["affine_trivial"]:
        common["gvec"] = plan["gvec"]
        common["bvec"] = plan["bvec"]

    B_core = plan["B_core"]
    in_maps = []
    for c in range(N_CORES):
        m = dict(common)
        if plan["n1"]:
            m["x1g"] = np.ascontiguousarray(
                plan["x1g"][:, c * B_core : (c + 1) * B_core]
            )
        if plan["n2"]:
            m["x2g"] = np.ascontiguousarray(
                plan["x2g"][:, c * B_core : (c + 1) * B_core]
            )
        in_maps.append(m)

    trace = os.environ.get("BASS_KERNEL_TRACE", "") == "1"
    kwargs = {}
    if trace:
        from concourse import bass_utils

        bass_utils.upload_artifacts = lambda tmpdir: "local://" + tmpdir
        kwargs["trace"] = True
        if os.environ.get("BASS_KERNEL_TRACE_ALL", "") == "1":
            kwargs["trace_cores"] = list(range(N_CORES))

    from concourse.bass_utils import run_bass_kernel_spmd

    res = run_bass_kernel_spmd(nc, in_maps, list(range(N_CORES)), **kwargs)
    if trace:
        global LAST_RESULTS
        LAST_RESULTS = res
        LAST_EXEC_TIME_NS = res.exec_time_ns
        LAST_TRACE_PATH = (
            res.instructions_and_trace[1] if res.instructions_and_trace else None
        )

    return np.concatenate([res.results[c]["y"] for c in range(N_CORES)], 0)


# revision 23
# speedup vs baseline: 1.4480x; 1.0388x over previous
"""Trainium2 Bass kernel for CompactKroneckerFusion.

Math: out = relu(LN((x1@S1 * x2@S2) @ W + b)), where S1/S2 are count-sketch
matrices (exactly one +-1 per row). The product (x1@S1)*(x2@S2) is nonzero
only on sketch buckets hit by BOTH sketches (~117 of 8192 for these shapes),
so the whole computation collapses to small gathers + tiny dense GEMMs:

  J      = {buckets hit by both sketches}            (|J| = nj)
  x1g    = x1 columns that land in J, transposed     [n1, B]
  A1     = (col -> bucket-in-J) +-1 scatter matrix   [n1, nj]
  sk1^T  = A1^T @ x1g                                [nj, B]   (PE matmul)
  ck^T   = sk1^T * sk2^T  (+ ones row for bias)      [nj+1, B] (DVE)
  h      = ck^T^T @ [W[J]; b]                        [B, OUT]  (PE matmul)
  out    = relu((h - mu) * rsqrt(var + eps) * gamma + beta)    (DVE + ACT)

Sharding: data-parallel over batch across 8 cores; A1/A2/W[J] replicated.
Host side only extracts indices / gathers columns (cheap, O(input size));
all FLOPs above run on-device.
"""

import os
import sys
from contextlib import ExitStack

import numpy as np

_REPO = "/opt/trn_rl_repo"
if _REPO not in sys.path:
    sys.path.insert(0, _REPO)

import concourse.bass as bass  # noqa: E402
import concourse.mybir as mybir  # noqa: E402
import concourse.tile as tile  # noqa: E402

N_CORES = 8
PMAX = 128  # partitions / max matmul K and M
NMAX = 512  # max matmul moving free dim (one PSUM bank of f32)
F32 = mybir.dt.float32
LN_EPS = 1e-5

# PE fp32 matmul runs at 4 cycles/row; float32r (same 32-bit storage,
# reduced-precision PE path) runs at 1 cycle/row for moving dim >= 256.
# All matmul operands (x panels, A scatter matrices, Wg, ck) are declared
# float32r end-to-end; PSUM accumulation stays fp32.
MM_DT = os.environ.get("BASS_KERNEL_MM_DT", "float32r")
XDT = mybir.dt.float32r if MM_DT == "float32r" else mybir.dt.float32

LAST_EXEC_TIME_NS = None
LAST_TRACE_PATH = None
LAST_RESULTS = None


# ---------------------------------------------------------------------------
# Toolchain workaround: this walrus build rejects instructions carrying more
# than one sync wait ("Too many sync wait commands").  After Tile lowering,
# hoist surplus waits onto same-engine NoOps inserted immediately before the
# owning instruction — the engine stalls on the carriers first, so ordering
# semantics are preserved.
# ---------------------------------------------------------------------------
def _split_multi_waits(nc, max_waits=1):
    n_split = 0
    for f in nc.m.functions:
        for blk in f.blocks:
            insts = blk.instructions
            out = []
            for inst in insts:
                si = inst.sync_info
                waits = list(si.on_wait) if si is not None and si.on_wait else []
                if len(waits) > max_waits:
                    extra = waits[: len(waits) - max_waits]
                    si.on_wait[:] = waits[len(waits) - max_waits :]
                    for k, w in enumerate(extra):
                        nop = mybir.InstNoOp(
                            name=f"{inst.name}-wc{k}", ins=[], outs=[]
                        )
                        nop.engine = inst.engine
                        nop.sync_info = mybir.SyncInfo(on_wait=[w], on_update=[])
                        out.append(nop)
                        n_split += 1
                out.append(inst)
            insts[:] = out
    return n_split


# ---------------------------------------------------------------------------
# Host-side restructuring
# ---------------------------------------------------------------------------
def _extract_sketch(S):
    """Count-sketch matrix -> (bucket index, sign) per input dim."""
    S = np.asarray(S, dtype=np.float32)
    idx = np.abs(S).argmax(1).astype(np.int64)
    s = S[np.arange(S.shape[0]), idx]
    return idx, s


def _plan_side(idx, s, pos, jchunks):
    """Group contributing input columns by J-chunk and split into K-subchunks.

    Returns (src_cols, dest_rows, per-chunk subchunk descriptors, n_rows).
    Each subchunk: (row_offset_in_xg, A_matrix [len_sub, njc]).  Chunk row
    bases are 32-aligned so matmul partition reads stay legal.
    """
    keep = (s != 0) & (pos[idx] >= 0)
    cols = np.where(keep)[0]
    p = pos[idx[cols]]
    chunk_of = p // PMAX
    order = np.lexsort((cols, chunk_of))
    cols = cols[order]
    p = p[order]
    sg = s[cols]

    per_chunk = []
    dest_rows = np.empty(len(cols), np.int64)
    row_base = 0
    for ci, (c0, njc) in enumerate(jchunks):
        ccols = np.where(chunk_of[order] == ci)[0]  # positions within `cols`
        dest_rows[ccols] = row_base + np.arange(len(ccols))
        subs = []
        for s0 in range(0, len(ccols), PMAX):
            sel = ccols[s0 : s0 + PMAX]
            A = np.zeros((len(sel), njc), np.float32)
            A[np.arange(len(sel)), p[sel] - c0] = sg[sel]
            subs.append((row_base + s0, A))
        per_chunk.append(subs)
        row_base += len(ccols)
        row_base = (row_base + 31) // 32 * 32
    return cols, dest_rows, per_chunk, max(row_base, 1)


def _prepare(x1, x2, S1, S2, W, b, ln_gamma, ln_beta):
    x1 = np.ascontiguousarray(np.asarray(x1, np.float32))
    x2 = np.ascontiguousarray(np.asarray(x2, np.float32))
    W = np.asarray(W, np.float32)
    b = np.asarray(b, np.float32)
    ln_gamma = np.asarray(ln_gamma, np.float32)
    ln_beta = np.asarray(ln_beta, np.float32)

    B = x1.shape[0]
    OUT = W.shape[1]
    SK = S1.shape[1]
    assert OUT <= NMAX, "OUT dim > 512 not supported by this kernel"
    assert B % (N_CORES * PMAX) == 0

    idx1, s1 = _extract_sketch(S1)
    idx2, s2 = _extract_sketch(S2)
    J = np.intersect1d(idx1[s1 != 0], idx2[s2 != 0])
    nj = len(J)
    pos = np.full(SK, -1, np.int64)
    pos[J] = np.arange(nj)

    jchunks = [(c0, min(PMAX, nj - c0)) for c0 in range(0, nj, PMAX)]

    cols1, dest1, sub1, nr1 = _plan_side(idx1, s1, pos, jchunks)
    cols2, dest2, sub2, nr2 = _plan_side(idx2, s2, pos, jchunks)

    x1g = np.zeros((nr1, B), np.float32)
    if len(cols1):
        x1g[dest1] = x1[:, cols1].T
    x2g = np.zeros((nr2, B), np.float32)
    if len(cols2):
        x2g[dest2] = x2[:, cols2].T

    # W rows for each chunk; bias folded in as a ones-row contraction on the
    # last chunk (or as its own chunk when there's no room / no buckets).
    # Compute-engine SBUF writes must start at a 32-aligned partition, so a
    # bias-carrying chunk is padded to 128 rows: ck rows [96:128) are preset
    # to 1.0 (the product overwrites rows up to njc), and Wg rows beyond njc
    # are zero except the bias row at njc — the spurious ones hit zero rows.
    chunks = []
    for ci, (c0, njc) in enumerate(jchunks):
        chunks.append(
            {"njc": njc, "has_bias": False, "nrows": njc,
             "Wg": W[J[c0 : c0 + njc], :], "sub1": sub1[ci], "sub2": sub2[ci]}
        )
    if not chunks or chunks[-1]["njc"] == PMAX:
        chunks.append(
            {"njc": 0, "has_bias": True, "nrows": 0,
             "Wg": np.zeros((0, OUT), np.float32), "sub1": [], "sub2": []}
        )
    ch = chunks[-1]
    ch["has_bias"] = True
    pad = np.zeros((PMAX - ch["njc"], OUT), np.float32)
    pad[0] = b
    ch["Wg"] = np.concatenate([ch["Wg"], pad], 0)
    ch["nrows"] = PMAX
    for ch in chunks:
        ch["Wg"] = np.ascontiguousarray(ch["Wg"], np.float32)

    affine_trivial = bool(np.all(ln_gamma == 1.0) and np.all(ln_beta == 0.0))

    return {
        "B": B,
        "OUT": OUT,
        "B_core": B // N_CORES,
        "n1": x1g.shape[0],
        "n2": x2g.shape[0],
        "x1g": x1g,
        "x2g": x2g,
        "chunks": chunks,
        "affine_trivial": affine_trivial,
        "gvec": np.ascontiguousarray(ln_gamma[None, :]),
        "bvec": np.ascontiguousarray(ln_beta[None, :]),
    }


# ---------------------------------------------------------------------------
# Device program
# ---------------------------------------------------------------------------
def _build_program(plan):
    B_core = plan["B_core"]
    OUT = plan["OUT"]
    chunks = plan["chunks"]
    BT = NMAX if B_core % NMAX == 0 else PMAX
    assert B_core % BT == 0 and BT % PMAX == 0
    n_t = B_core // BT
    n_m = BT // PMAX

    nc = bass.Bass()

    x1g_d = (
        nc.dram_tensor("x1g", [plan["n1"], B_core], XDT, kind="ExternalInput")
        if plan["n1"]
        else None
    )
    x2g_d = (
        nc.dram_tensor("x2g", [plan["n2"], B_core], XDT, kind="ExternalInput")
        if plan["n2"]
        else None
    )
    a1_d, a2_d, wg_d = [], [], []
    for ci, ch in enumerate(chunks):
        a1_d.append(
            [
                nc.dram_tensor(f"A1_{ci}_{si}", list(A.shape), XDT, kind="ExternalInput")
                for si, (_, A) in enumerate(ch["sub1"])
            ]
        )
        a2_d.append(
            [
                nc.dram_tensor(f"A2_{ci}_{si}", list(A.shape), XDT, kind="ExternalInput")
                for si, (_, A) in enumerate(ch["sub2"])
            ]
        )
        wg_d.append(
            nc.dram_tensor(f"Wg_{ci}", list(ch["Wg"].shape), XDT, kind="ExternalInput")
        )
    if not plan["affine_trivial"]:
        g_d = nc.dram_tensor("gvec", [1, OUT], F32, kind="ExternalInput")
        be_d = nc.dram_tensor("bvec", [1, OUT], F32, kind="ExternalInput")
    y_d = nc.dram_tensor("y", [B_core, OUT], F32, kind="ExternalOutput")

    with tile.TileContext(nc) as tc, ExitStack() as ctx:
        consts = ctx.enter_context(tc.tile_pool(name="consts", bufs=1))
        xin = ctx.enter_context(tc.tile_pool(name="xin", bufs=4))
        ckp = ctx.enter_context(tc.tile_pool(name="ck", bufs=2))
        pss = ctx.enter_context(tc.tile_pool(name="pss", bufs=2, space="PSUM"))
        psh = ctx.enter_context(tc.tile_pool(name="psh", bufs=4, space="PSUM"))
        stat = ctx.enter_context(tc.tile_pool(name="stat", bufs=4))
        outp = ctx.enter_context(tc.tile_pool(name="outp", bufs=4))

        a1_sb, a2_sb, wg_sb = [], [], []
        for ci, ch in enumerate(chunks):
            row1 = []
            for si, (_, A) in enumerate(ch["sub1"]):
                t = consts.tile(list(A.shape), XDT, tag=f"A1_{ci}_{si}")
                nc.sync.dma_start(out=t[:], in_=a1_d[ci][si][:])
                row1.append(t)
            a1_sb.append(row1)
            row2 = []
            for si, (_, A) in enumerate(ch["sub2"]):
                t = consts.tile(list(A.shape), XDT, tag=f"A2_{ci}_{si}")
                nc.scalar.dma_start(out=t[:], in_=a2_d[ci][si][:])
                row2.append(t)
            a2_sb.append(row2)
            t = consts.tile(list(ch["Wg"].shape), XDT, tag=f"Wg_{ci}")
            nc.gpsimd.dma_start(out=t[:], in_=wg_d[ci][:])
            wg_sb.append(t)
        eps_t = consts.tile([PMAX, 1], F32, tag="eps")
        nc.vector.memset(eps_t[:], LN_EPS)
        if not plan["affine_trivial"]:
            g_sb = consts.tile([PMAX, OUT], F32, tag="gamma")
            nc.gpsimd.dma_start(out=g_sb[:], in_=g_d[:].to_broadcast([PMAX, OUT]))
            be_sb = consts.tile([PMAX, OUT], F32, tag="beta")
            nc.gpsimd.dma_start(out=be_sb[:], in_=be_d[:].to_broadcast([PMAX, OUT]))

        # x panels stream in pieces: DMA completion latency is roughly
        # 1.4 us + 2x the transfer slice, so the first pieces are small to
        # let the PE start early, then widen.  x1 rides the sync HWDGE
        # ring, x2 the gpsimd SWDGE queue; the scalar ring only carries A2
        # up front so the ACT engine never stalls on transfers mid-kernel.
        # Output stores all ride the (otherwise idle) sync ring.
        def pieces_for(ti):
            if ti == 0 and BT >= 512:
                return [(0, 128), (128, 128), (256, 256)]
            return [(p0, min(256, BT - p0)) for p0 in range(0, BT, 256)]

        any_buckets = any(ch["njc"] > 0 for ch in chunks)
        for ti in range(n_t):
            cks = []
            for ci, ch in enumerate(chunks):
                ck = ckp.tile([ch["nrows"], BT], XDT, tag=f"ck{ci}")
                if ch["has_bias"]:
                    ones0 = (ch["njc"] // 32) * 32
                    nc.gpsimd.memset(ck[ones0:PMAX, :].bitcast(F32), 1.0)
                cks.append(ck)
            for pi, (poff, pw) in enumerate(pieces_for(ti)):
                if not any_buckets:
                    break
                psl = bass.ds(ti * BT + poff, pw)
                x1t = xin.tile([plan["n1"], pw], XDT, tag="x1")
                nc.sync.dma_start(out=x1t[:], in_=x1g_d[:, psl])
                x2t = xin.tile([plan["n2"], pw], XDT, tag="x2")
                nc.gpsimd.dma_start(out=x2t[:], in_=x2g_d[:, psl])
                for ci, ch in enumerate(chunks):
                    njc = ch["njc"]
                    if njc == 0:
                        continue
                    ps1 = pss.tile([njc, pw], F32, tag=f"ps1_{ci}")
                    for si, (r0, A) in enumerate(ch["sub1"]):
                        nc.tensor.matmul(
                            ps1[:],
                            a1_sb[ci][si][:],
                            x1t[r0 : r0 + A.shape[0], :],
                            start=(si == 0),
                            stop=(si == len(ch["sub1"]) - 1),
                        )
                    ps2 = pss.tile([njc, pw], F32, tag=f"ps2_{ci}")
                    for si, (r0, A) in enumerate(ch["sub2"]):
                        nc.tensor.matmul(
                            ps2[:],
                            a2_sb[ci][si][:],
                            x2t[r0 : r0 + A.shape[0], :],
                            start=(si == 0),
                            stop=(si == len(ch["sub2"]) - 1),
                        )
                    sk1 = ckp.tile([njc, pw], F32, tag=f"sk1_{ci}")
                    nc.scalar.copy(sk1[:], ps1[:])
                    nc.vector.tensor_mul(
                        cks[ci][0:njc, bass.ds(poff, pw)], sk1[:], ps2[:]
                    )

            # h = ck^T @ Wg for the n_m row-tiles, then a batched LN
            # epilogue: stats for all n_m tiles are aggregated first so the
            # sqrt/reciprocal scalar chain runs once per iteration.
            phs = []
            stats4 = stat.tile([PMAX, n_m, 6], F32, tag="stats4")
            mv4 = stat.tile([PMAX, n_m, 2], F32, tag="mv4")
            for mi in range(n_m):
                msl = bass.ts(mi, PMAX)
                ph = psh.tile([PMAX, OUT], F32, tag="ph")
                for ci, ch in enumerate(chunks):
                    nc.tensor.matmul(
                        ph[:],
                        cks[ci][:, msl],
                        wg_sb[ci][:],
                        start=(ci == 0),
                        stop=(ci == len(chunks) - 1),
                    )
                nc.vector.bn_stats(stats4[:, mi, :], ph[:])
                nc.vector.bn_aggr(mv4[:, mi, :], stats4[:, mi, :])
                phs.append(ph)
            std4 = stat.tile([PMAX, n_m], F32, tag="std4")
            nc.scalar.activation(
                std4[:],
                mv4[:, :, 1],
                mybir.ActivationFunctionType.Sqrt,
                bias=eps_t[:],
            )
            rstd4 = stat.tile([PMAX, n_m], F32, tag="rstd4")
            nc.vector.reciprocal(rstd4[:], std4[:])
            nmr4 = stat.tile([PMAX, n_m], F32, tag="nmr4")
            nc.vector.tensor_tensor(
                out=nmr4[:],
                in0=mv4[:, :, 0],
                in1=rstd4[:],
                op=mybir.AluOpType.mult,
            )
            nc.vector.tensor_scalar_mul(nmr4[:], nmr4[:], -1.0)
            for mi in range(n_m):
                out_t = outp.tile([PMAX, OUT], F32, tag="out")
                if plan["affine_trivial"]:
                    nc.scalar.activation(
                        out_t[:],
                        phs[mi][:],
                        mybir.ActivationFunctionType.Relu,
                        bias=nmr4[:, mi : mi + 1],
                        scale=rstd4[:, mi : mi + 1],
                    )
                else:
                    tmp = outp.tile([PMAX, OUT], F32, tag="tmp")
                    nc.scalar.activation(
                        tmp[:],
                        phs[mi][:],
                        mybir.ActivationFunctionType.Identity,
                        bias=nmr4[:, mi : mi + 1],
                        scale=rstd4[:, mi : mi + 1],
                    )
                    nc.vector.tensor_mul(tmp[:], tmp[:], g_sb[:])
                    nc.vector.tensor_add(tmp[:], tmp[:], be_sb[:])
                    nc.scalar.activation(
                        out_t[:], tmp[:], mybir.ActivationFunctionType.Relu
                    )
                nc.sync.dma_start(
                    out=y_d[ti * BT + mi * PMAX :][:PMAX, :], in_=out_t[:]
                )

    return nc


# ---------------------------------------------------------------------------
# Entry point
# ---------------------------------------------------------------------------
def kernel(x1, x2, S1, S2, W, b, ln_gamma, ln_beta):
    global LAST_EXEC_TIME_NS, LAST_TRACE_PATH
    plan = _prepare(x1, x2, S1, S2, W, b, ln_gamma, ln_beta)
    nc = _build_program(plan)
    _split_multi_waits(nc)

    common = {}
    for ci, ch in enumerate(plan["chunks"]):
        for si, (_, A) in enumerate(ch["sub1"]):
            common[f"A1_{ci}_{si}"] = A
        for si, (_, A) in enumerate(ch["sub2"]):
            common[f"A2_{ci}_{si}"] = A
        common[f"Wg_{ci}"] = ch["Wg"]
    if not plan["affine_trivial"]:
        common["gvec"] = plan["gvec"]
        common["bvec"] = plan["bvec"]

    B_core = plan["B_core"]
    in_maps = []
    for c in range(N_CORES):
        m = dict(common)
        if plan["n1"]:
            m["x1g"] = np.ascontiguousarray(
                plan["x1g"][:, c * B_core : (c + 1) * B_core]
            )
        if plan["n2"]:
            m["x2g"] = np.ascontiguousarray(
                plan["x2g"][:, c * B_core : (c + 1) * B_core]
            )
        in_maps.append(m)

    trace = os.environ.get("BASS_KERNEL_TRACE", "") == "1"
    kwargs = {}
    if trace:
        from concourse import bass_utils

        bass_utils.upload_artifacts = lambda tmpdir: "local://" + tmpdir
        kwargs["trace"] = True
        if os.environ.get("BASS_KERNEL_TRACE_ALL", "") == "1":
            kwargs["trace_cores"] = list(range(N_CORES))

    from concourse.bass_utils import run_bass_kernel_spmd

    res = run_bass_kernel_spmd(nc, in_maps, list(range(N_CORES)), **kwargs)
    if trace:
        global LAST_RESULTS
        LAST_RESULTS = res
        LAST_EXEC_TIME_NS = res.exec_time_ns
        LAST_TRACE_PATH = (
            res.instructions_and_trace[1] if res.instructions_and_trace else None
        )

    return np.concatenate([res.results[c]["y"] for c in range(N_CORES)], 0)
